# revision 28
# baseline (speedup 1.0000x reference)
"""MinGRU kernel for Trainium2 (8 NeuronCores, Bass/Tile).

Reference computation (B=4, L=8192, D=512, fp32):
    gates = sigmoid(x @ Wg.T + bg)
    cands = tanh(x @ Wc.T + bc)
    h_t   = (1 - g_t) * h_{t-1} + g_t * c_t   (scan along L, h_0 = 0)

Sharding: core c -> (batch b = c//2, channel half = c%2). Each core computes
its batch's full L range for 256 of the 512 output channels; the scan along L
is per (b, channel) so no cross-core communication is needed.

Layout: host pre-transposes x[b] to [D, L] and weights to [D, 256] (lhsT) so
every device DMA is fully contiguous. On device, matmuls keep channels on
partitions and tokens on the free axis, which is exactly the layout
tensor_tensor_scan needs (recurrence runs along the free dim). The scan uses
    state = (a * state) - bneg,   a = sigmoid(-z_g - bg) = 1 - g,
    bneg = (a - 1) * c = -g * c
so a single scalar_tensor_tensor op feeds the scan. Output is [256, L] per
core; the host reassembles [B, L, D].

Precision: x and W feed the PE as float32r (reduced-precision single-pass
fp32 matmul, 1 row/cycle like bf16 for moving dims >= 256); PSUM accumulation
is fp32. The activation outputs a/c, the scan operands, and the stored h are
fp16 (the scan's internal state stays fp32 per the ISA); h is written to HBM
as fp16 and upcast on the host, halving output DMA bytes. End-to-end max
relative error ~1e-3.
"""

import os
import sys

sys.path.insert(0, "/opt/trn_rl_repo")

import numpy as np

import concourse.bacc as bacc
import concourse.bass as bass
import concourse.mybir as mybir
from concourse.bass_utils import run_bass_kernel_spmd
from concourse.tile import TileContext

B, L, D = 4, 8192, 512
NCORES = 8
EH = D // 2          # output channels per core
NET = EH // 128      # e-tiles per core (2)
NDC = D // 128       # contraction chunks (4)
NSUB = 512           # matmul moving free dim (= 1 fp32 PSUM bank)
# Token segments: small head segments start the PE/scan pipeline early, small
# tail segments shrink the post-last-DMA drain.
SEGS = [256, 256, 512, 1024, 1024, 1024, 1024, 1024, 1024, 512, 256, 256]
assert sum(SEGS) == L

FP32 = mybir.dt.float32
F32R = mybir.dt.float32r
F16 = mybir.dt.float16
_last_results = None


def build_nc() -> bass.Bass:
    # Bacc (not plain Bass): its compile() runs move_matmul_waits_to_ldweights
    # and generate_event_semaphores, which split multi-sem waits to satisfy the
    # TRN2 per-instruction wait-slot limits walrus enforces.
    nc = bacc.Bacc()

    xT = nc.dram_tensor("xT", [D, L], F32R, kind="ExternalInput")
    wgT = nc.dram_tensor("wgT", [D, EH], F32R, kind="ExternalInput")
    wcT = nc.dram_tensor("wcT", [D, EH], F32R, kind="ExternalInput")
    # biases packed [128, 4]: cols 0..1 = bg per e-tile, 2..3 = bc per e-tile
    bias = nc.dram_tensor("bias", [128, 2 * NET], FP32, kind="ExternalInput")
    h = nc.dram_tensor("h", [EH, L], F16, kind="ExternalOutput")

    op = mybir.AluOpType
    act = mybir.ActivationFunctionType

    with TileContext(nc) as tc:
        with (
            tc.tile_pool(name="consts", bufs=1) as consts,
            tc.tile_pool(name="xpool", bufs=5) as xpool,
            tc.tile_pool(name="work", bufs=3) as work,
            tc.tile_pool(name="hpool", bufs=3) as hpool,
            tc.tile_pool(name="psum", bufs=2, space="PSUM") as psum,
        ):
            # Sync HWDGE queue order: wg -> x seg 0 -> wc -> x seg 1 -> ...
            # The first matmul group only needs wg + the first x segment, so
            # this starts the PE as early as possible. Biases ride the SWDGE
            # (gpsimd) queue.
            wg_sb = consts.tile([128, NDC, EH], F32R)
            wc_sb = consts.tile([128, NDC, EH], F32R)
            nc.sync.dma_start(wg_sb, wgT.rearrange("(c p) e -> p c e", p=128))
            x0_sb = xpool.tile([128, NDC, max(SEGS)], F32R, tag="x", name="x_0")[
                :, :, : SEGS[0]
            ]
            nc.sync.dma_start(x0_sb, xT[:, 0 : SEGS[0]].rearrange("(c p) l -> p c l", p=128))
            nc.sync.dma_start(wc_sb, wcT.rearrange("(c p) e -> p c e", p=128))

            bias_sb = consts.tile([128, 2 * NET], FP32)
            bgn_sb = consts.tile([128, NET], FP32)
            nc.gpsimd.dma_start(bias_sb, bias[:])
            nc.scalar.mul(bgn_sb, bias_sb[:, 0:NET], -1.0)
            bc_sb = bias_sb[:, NET : 2 * NET]

            carry = [None] * NET  # [128, 1] AP of the previous h column

            l0 = 0
            for t, lt in enumerate(SEGS):
                if t == 0:
                    x_sb = x0_sb
                else:
                    x_sb = xpool.tile([128, NDC, max(SEGS)], F32R, tag="x", name=f"x_{t}")[
                        :, :, :lt
                    ]
                    nc.sync.dma_start(
                        x_sb, xT[:, l0 : l0 + lt].rearrange("(c p) l -> p c l", p=128)
                    )
                for et in range(NET):
                    esl = slice(et * 128, (et + 1) * 128)
                    a_t = work.tile([128, max(SEGS)], F16, tag=f"a{et}", name=f"a{et}_{t}")[:, :lt]
                    c_t = work.tile([128, max(SEGS)], F16, tag=f"c{et}", name=f"c{et}_{t}")[:, :lt]
                    for n, n0 in enumerate(range(0, lt, NSUB)):
                        w = min(NSUB, lt - n0)
                        nsl = slice(n0, n0 + w)
                        # One 2-bank PSUM tile per (g, c) pair: [*, 0, :] = z_g,
                        # [*, 1, :] = z_c.
                        pz = psum.tile(
                            [128, 2, NSUB], FP32, tag=f"pz{et}", name=f"pz{et}_{t}_{n}"
                        )
                        for dc in range(NDC):
                            nc.tensor.matmul(
                                pz[:, 0, :w],
                                wg_sb[:, dc, esl],
                                x_sb[:, dc, nsl],
                                start=(dc == 0),
                                stop=(dc == NDC - 1),
                            )
                        for dc in range(NDC):
                            nc.tensor.matmul(
                                pz[:, 1, :w],
                                wc_sb[:, dc, esl],
                                x_sb[:, dc, nsl],
                                start=(dc == 0),
                                stop=(dc == NDC - 1),
                            )
                        # a = sigmoid(-(z_g + bg)) = 1 - g ; c = tanh(z_c + bc)
                        nc.scalar.activation(
                            a_t[:, nsl], pz[:, 0, :w], act.Sigmoid,
                            bias=bgn_sb[:, et : et + 1], scale=-1.0,
                        )
                        nc.scalar.activation(
                            c_t[:, nsl], pz[:, 1, :w], act.Tanh,
                            bias=bc_sb[:, et : et + 1], scale=1.0,
                        )
                    # bneg = (a - 1) * c = -g * c  (one DVE op, full segment)
                    bn_t = work.tile([128, max(SEGS)], F16, tag=f"b{et}", name=f"b{et}_{t}")[:, :lt]
                    nc.vector.scalar_tensor_tensor(bn_t, a_t, 1.0, c_t, op.subtract, op.mult)
                    # h = a * h_prev - bneg  (fp32 state in HW, fp16 storage)
                    h_t = hpool.tile([128, max(SEGS)], F16, tag=f"h{et}", name=f"h{et}_{t}")[:, :lt]
                    init = 0.0 if carry[et] is None else carry[et]
                    nc.vector.tensor_tensor_scan(h_t, a_t, bn_t, init, op.mult, op.subtract)
                    carry[et] = h_t[:, lt - 1 : lt]
                    # h writes on the SWDGE queue: keeps the sync HWDGE queue
                    # a pure x-feed.
                    nc.gpsimd.dma_start(h[et * 128 : (et + 1) * 128, l0 : l0 + lt], h_t)
                l0 += lt
    return nc


def _in_maps(x, Wg, bg, Wc, bc):
    maps = []
    xT = {}
    for c in range(NCORES):
        b, eh = c // 2, c % 2
        e0 = eh * EH
        if b not in xT:
            xT[b] = np.ascontiguousarray(x[b].T)
        bias_pack = np.concatenate(
            [
                bg[e0 : e0 + EH].reshape(NET, 128).T,
                bc[e0 : e0 + EH].reshape(NET, 128).T,
            ],
            axis=1,
        )
        maps.append(
            {
                "xT": xT[b],
                "wgT": np.ascontiguousarray(Wg[e0 : e0 + EH].T),
                "wcT": np.ascontiguousarray(Wc[e0 : e0 + EH].T),
                "bias": np.ascontiguousarray(bias_pack),
            }
        )
    return maps


def kernel(x, Wg, bg, Wc, bc):
    global _last_results
    x = np.asarray(x, dtype=np.float32)
    Wg = np.asarray(Wg, dtype=np.float32)
    bg = np.asarray(bg, dtype=np.float32)
    Wc = np.asarray(Wc, dtype=np.float32)
    bc = np.asarray(bc, dtype=np.float32)

    nc = build_nc()
    if not nc.is_finalized():
        nc.finalize()
    res = run_bass_kernel_spmd(
        nc,
        _in_maps(x, Wg, bg, Wc, bc),
        list(range(NCORES)),
        tmpdir=os.environ.get("KERNEL_TMPDIR"),
    )
    _last_results = res

    out = np.empty((B, L, D), dtype=np.float32)
    for b in range(B):
        hb = np.concatenate(
            [res.results[2 * b]["h"], res.results[2 * b + 1]["h"]], axis=0
        ).astype(np.float32)
        out[b] = hb.T
    return out
